# revision 73
# baseline (speedup 1.0000x reference)
"""Trainium2 Bass kernel for the sum-product "knowledge layer" network.

Computation (see problem reference):
  h0 = encode(x): 8194-row table [-inf, 0, pos0, neg0, ...] with pos = x
       (log-probs), neg = log(1 - exp(x)), per batch column.
  4 alternating layers: gather rows by ptrs, segment-reduce over contiguous
  fanin groups (fanin-4 sum "product" layers, fanin-2 logsumexp "sum" layers).

Strategy (pure batch data-parallelism, 8 NeuronCores, SBUF-resident):
  - Shard the 512 batch columns 8 ways -> 64 columns per core.
  - Batch-on-partition layout: every table lives in SBUF as [128, n_rows]
    f32 where channel c holds batch column c%64 (two identical copies).
  - Gathers use the gpsimd ap_gather instruction (SBUF->SBUF, per-16-
    partition-group index lists): channels 0-63 gather edge-set A while
    64-127 gather edge-set B, so each free slot delivers 2 edges.  No DMA
    descriptors per edge, no DRAM round trips between layers.
  - Layer fusion: t1 rows are consumed on average once by layer 1 and t3
    rows once by layer 3, so layers (0,1) and (2,3) are fused.  The host
    composes the pointer chains: one device gather phase fetches, per fused
    output group g, the 8 t0-rows {ptr0[4*ptr1[2g+j]+k]}, the device then
    computes lse(sum4, sum4).  Phase 2 repeats the pattern from t2.
  - Dead-row elimination: the ptr3->ptr2 composition references only ~72%
    of t2 rows; phase 1 computes just those (compacted by the host).
  - Early/late phase-2 split: phase-2 groups whose rows are all produced by
    the non-drain phase-1 chunks gather while the drain chunk reduces; out
    rows are emitted permuted and the host inverse-permutes for free.
  - Encode avoids interleaving: t0 = [2 const rows | pos block | neg block]
    and the host remaps ptr0 accordingly; x DMA-loads straight into the pos
    block, Act writes exp/log1mexp into the neg block.
  - The exp+ln activation-table patch keeps both functions in one act set
    (one LoadActFuncSet instead of a 1.3us reload per switch).
"""

import numpy as np

P = 128
B = 64  # batch columns per core
NCORES = 8
N_VARS = 4096
BATCH = 512
TAB0 = 2 * N_VARS + 2  # 8194
OUT_SIZES = [16384, 8192, 4096, 2048]
FANINS = [4, 2, 4, 2]
N_MID = 8192  # t2 rows before dead-row elimination
N_OUT = 2048
# Phase 2 references only ~72% of t2 rows (the ptr3->ptr2 composition skips
# the rest), so phase 1 computes just the live rows.  Phase 2 is further
# split: groups whose 8 rows all land in the first two phase-1 chunks gather
# early (overlapping the phase-1 drain); the host inverse-permutes output
# rows.  The default layout below matches the fixed reference inputs;
# host_prep recomputes it for any other inputs.
DEFAULT_PLAN = ((2052, 2052, 1804), 1024, 1024)


def _plan(n_used):
    """Phase-1 chunk sizes (group counts) for n_used live t2 rows.  Chunk
    count keeps each ap_gather instruction's slot count near the 8194-entry
    t0 floor; sizes are multiples of 4 so slot counts stay 16-aligned.  The
    final (drain) chunk is smaller: its reduce runs serially after the last
    gather, so shrinking it shortens that tail while the earlier chunks stay
    above the gather floor."""
    floor_g = -(-8194 // 16) * 4  # 2052: smallest group count >= t0 floor
    n_chunks = max(1, -(-n_used * 4 // 8192))
    gpc = -(-n_used // (4 * n_chunks)) * 4
    if n_chunks >= 2:
        last = -(-(n_used - floor_g * (n_chunks - 1)) // 4) * 4
        if 0 < last <= floor_g:
            return (floor_g,) * (n_chunks - 1) + (last,)
    return (gpc,) * n_chunks


def _pack_early(g2, e_rows):
    """Greedily select phase-2 groups whose row sets jointly fit in the
    first e_rows compacted t2 rows (the output of the non-drain phase-1
    chunks).  Returns (early group ids, late group ids, early row list)."""
    order = np.argsort([np.unique(r).size for r in g2], kind="stable")
    sel = np.zeros(len(g2), bool)
    early: set = set()
    for gi in order:
        new_rows = set(g2[gi]) - early
        if len(early) + len(new_rows) <= e_rows:
            early |= new_rows
            sel[gi] = True
    return np.where(sel)[0], np.where(~sel)[0], np.array(sorted(early), np.int64)


def wrap128(list_a, list_b):
    """Pack two per-half edge lists (len S each) into the ap_gather int16
    index layout [128, S//16]: position j of a group's list sits at
    [16*grp + j%16, j//16]; groups 0-3 share list A, 4-7 share list B."""
    a = np.asarray(list_a, np.int64)
    b = np.asarray(list_b, np.int64)
    assert a.size == b.size and a.size % 16 == 0
    wa = np.tile(a.reshape(-1, 16).T, (4, 1))
    wb = np.tile(b.reshape(-1, 16).T, (4, 1))
    w = np.concatenate([wa, wb], axis=0)
    assert w.min() >= 0 and w.max() < 2**15
    return np.ascontiguousarray(w.astype(np.int16))


def _patch_act_tables():
    """Make the combined exp+ln activation-function set the only candidate
    for Exp and Ln so the compiler emits a single LoadActFuncSet instead of
    ping-ponging between the exp-only and ln-only sets (1283ns per reload).
    Set ids (dict order) are preserved; the combined set genuinely contains
    both functions, so device behavior is unchanged."""
    import concourse.bacc as bacc
    import concourse.hw_specs as hws
    import concourse.mybir as mybir

    Act = mybir.ActivationFunctionType
    orig = hws.get_activation_tables

    def patched(arch):
        tabs = dict(orig(arch))
        out = {}
        for name, s in tabs.items():
            s2 = set(s)
            if name != "natural_log_exp_and_others":
                s2.discard(Act.Exp)
                s2.discard(Act.Ln)
            out[name] = s2
        return out

    bacc.get_activation_tables = patched


def build_nc(chunks, n2a, n2b):
    import concourse.bacc as bacc
    import concourse.mybir as mybir
    import concourse.tile as tile

    _patch_act_tables()

    f32 = mybir.dt.float32
    i16 = mybir.dt.int16
    Alu = mybir.AluOpType
    Act = mybir.ActivationFunctionType

    u_pad = sum(chunks)  # t2 rows incl. padding
    n_chunks = len(chunks)
    slots1 = [4 * gc for gc in chunks]  # phase-1 gather slots per chunk
    e_rows = sum(chunks[:-1])  # rows ready before the drain chunk
    slots2a, slots2b = 4 * n2a, 4 * n2b
    nc = bacc.Bacc(
        "TRN2",
        target_bir_lowering=False,
        debug=False,
        # no SWDGE DMAs in this kernel; shrink the descriptor carveout
        dynamic_dma_scratch_size=2048,
    )
    xv_in = nc.dram_tensor("xv", [P, N_VARS], f32, kind="ExternalInput")
    idx1_in = [
        nc.dram_tensor(f"idx1_{c}", [P, slots1[c] // 16], i16, kind="ExternalInput")
        for c in range(n_chunks)
    ]
    idx2a_in = nc.dram_tensor("idx2a", [P, slots2a // 16], i16, kind="ExternalInput")
    idx2b_in = nc.dram_tensor("idx2b", [P, slots2b // 16], i16, kind="ExternalInput")
    out_d = nc.dram_tensor("out", [B, n2a + n2b], f32, kind="ExternalOutput")

    with tile.TileContext(nc) as tc:
        with (
            tc.tile_pool(name="tabs", bufs=1) as tabs,
            tc.tile_pool(name="gp", bufs=2) as gp,
            tc.tile_pool(name="tp", bufs=2) as tp,
            tc.tile_pool(name="ix", bufs=1) as ixp,
        ):
            t0 = tabs.tile([P, TAB0], f32, tag="t0")
            t2 = tabs.tile([P, u_pad], f32, tag="t2")

            # --- encode: t0 = [0, 0 | pos_0..pos_4095 | neg_0..neg_4095] ---
            # x loads go first (the encode chain is the critical-path head);
            # halves let Exp start while the second half is still in flight.
            nq = 4
            q = N_VARS // nq
            nc.vector.memset(t0[:][:, 0:2], 0.0)
            for h in range(nq):
                nc.sync.dma_start(
                    t0[:][:, 2 + q * h : 2 + q * (h + 1)],
                    xv_in[:][:, q * h : q * (h + 1)],
                )
            ix1 = [
                ixp.tile([P, slots1[c] // 16], i16, tag=f"i{c}", name=f"ix1_{c}")
                for c in range(n_chunks)
            ]
            ix2a = ixp.tile([P, slots2a // 16], i16, tag="ix2a")
            ix2b = ixp.tile([P, slots2b // 16], i16, tag="ix2b")
            for c in range(n_chunks):
                nc.sync.dma_start(ix1[c][:], idx1_in[c][:])
            nc.sync.dma_start(ix2a[:], idx2a_in[:])
            nc.sync.dma_start(ix2b[:], idx2b_in[:])
            for h in range(nq):
                et = tp.tile([P, q], f32, tag="et", name=f"et{h}")
                pos = t0[:][:, 2 + q * h : 2 + q * (h + 1)]
                neg = t0[:][:, 2 + N_VARS + q * h : 2 + N_VARS + q * (h + 1)]
                nc.scalar.activation(et[:], pos, Act.Exp)
                nc.scalar.activation(neg, et[:], Act.Ln, scale=-1.0, bias=1.0)

            def reduce8_lse(g, dst, row0, rows_half, replicate, subs=2, pool_u=(), tail=False):
                """g [128, 8*rows_half]: per channel-half, rows_half fused
                groups of 8 slots [a0..a3, b0..b3]; writes
                lse(a0+..+a3, b0+..+b3) to dst rows [row0, row0+2*rows_half)
                (half A from channels 0-63, half B from 64-127).  dst may be
                an SBUF table tile or a DRAM [64, n] output.  With
                replicate=True both channel copies of dst get every row (DMA
                placement copies); otherwise each half lands only on its own
                channels."""
                assert rows_half % subs == 0, (rows_half, subs)
                sg = rows_half // subs  # groups per sub-chunk
                for s in range(subs):
                    gs = (
                        g[:][:, 8 * sg * s : 8 * sg * (s + 1)]
                        .rearrange("p (h k) -> p h k", k=8)
                    )
                    u = tp.tile([P, sg, 4], f32, tag="u", name=f"u{row0}_{s}")
                    # when the Pool engine has no gathers left (drain/phase 2)
                    # it takes a sub-chunk's first-stage add off the DVE chain
                    eng = nc.gpsimd if s in pool_u else nc.vector
                    eng.tensor_add(u[:], gs[:, :, 0::2], gs[:, :, 1::2])
                    w = tp.tile([P, sg, 2], f32, tag="w", name=f"w{row0}_{s}")
                    nc.vector.tensor_add(w[:], u[:][:, :, 0::2], u[:][:, :, 1::2])
                    m = tp.tile([P, sg], f32, tag="m", name=f"m{row0}_{s}")
                    mn = tp.tile([P, sg], f32, tag="n", name=f"n{row0}_{s}")
                    nc.vector.tensor_tensor(
                        m[:], w[:][:, :, 0], w[:][:, :, 1], op=Alu.max
                    )
                    nc.vector.tensor_tensor(
                        mn[:], w[:][:, :, 0], w[:][:, :, 1], op=Alu.min
                    )
                    nc.vector.tensor_tensor(mn[:], mn[:], m[:], op=Alu.subtract)
                    nc.scalar.activation(mn[:], mn[:], Act.Exp)
                    nc.scalar.activation(mn[:], mn[:], Act.Ln, bias=1.0)
                    ra = slice(row0 + sg * s, row0 + sg * (s + 1))
                    rb = slice(row0 + rows_half + sg * s, row0 + rows_half + sg * (s + 1))
                    if replicate and tail and s == subs - 1:
                        # last sub sits on the critical path into the next
                        # gather: write own-copy halves with DVE directly and
                        # cross-replicate with just 2 DMAs
                        nc.vector.tensor_add(dst[:][0:B, ra], m[:][0:B, :], mn[:][0:B, :])
                        nc.vector.tensor_add(dst[:][B:P, rb], m[:][B:P, :], mn[:][B:P, :])
                        nc.sync.dma_start(dst[:][B:P, ra], dst[:][0:B, ra])
                        nc.sync.dma_start(dst[:][0:B, rb], dst[:][B:P, rb])
                        continue
                    h = tp.tile([P, sg], f32, tag="h", name=f"h{row0}_{s}")
                    nc.vector.tensor_add(h[:], m[:], mn[:])
                    if replicate:
                        nc.sync.dma_start(dst[:][0:B, ra], h[:][0:B, :])
                        nc.sync.dma_start(dst[:][B:P, rb], h[:][B:P, :])
                        nc.sync.dma_start(dst[:][B:P, ra], h[:][0:B, :])
                        nc.sync.dma_start(dst[:][0:B, rb], h[:][B:P, :])
                    else:  # dst is the DRAM output [64, n]
                        nc.sync.dma_start(dst[:][:, ra], h[:][0:B, :])
                        nc.sync.dma_start(dst[:][:, rb], h[:][B:P, :])

            # --- phase 1: fused layers 0+1 -> live t2 rows ---
            g_free = max(max(slots1), slots2a, slots2b)
            row0 = 0
            for ci in range(n_chunks):
                g = gp.tile([P, g_free], f32, tag="g", name=f"g1_{ci}")
                nc.gpsimd.ap_gather(
                    g[:][:, : slots1[ci]],
                    t0[:].rearrange("p (n d) -> p n d", d=1),
                    ix1[ci][:],
                    P,
                    TAB0,
                    1,
                    slots1[ci],
                )
                hg = chunks[ci] // 2  # groups per half this chunk
                reduce8_lse(
                    g,
                    t2,
                    row0,
                    hg,
                    replicate=True,
                    pool_u=(1,) if ci == n_chunks - 1 else (),
                    tail=ci == n_chunks - 1,
                )
                row0 += chunks[ci]

            # --- phase 2: fused layers 2+3 -> out (early/late split) ---
            # early groups read only rows < e_rows, so their gather and
            # reduce overlap the drain chunk's reduce
            ga = gp.tile([P, g_free], f32, tag="g", name="g2a")
            nc.gpsimd.ap_gather(
                ga[:][:, :slots2a],
                t2[:][:, :e_rows].rearrange("p (n d) -> p n d", d=1),
                ix2a[:],
                P,
                e_rows,
                1,
                slots2a,
            )
            reduce8_lse(ga, out_d, 0, n2a // 2, replicate=False, pool_u=())
            gb = gp.tile([P, g_free], f32, tag="g", name="g2b")
            nc.gpsimd.ap_gather(
                gb[:][:, :slots2b],
                t2[:].rearrange("p (n d) -> p n d", d=1),
                ix2b[:],
                P,
                u_pad,
                1,
                slots2b,
            )
            reduce8_lse(gb, out_d, n2a, n2b // 2, replicate=False, pool_u=(0,))
    nc.compile()
    return nc


def host_prep(x, ptrs_list, seg_list, n_vars=N_VARS):
    """Host-side sharding + pointer-chain composition. Returns per-core
    input maps."""
    x = np.asarray(x, dtype=np.float32)
    p0, p1, p2, p3 = [np.asarray(p).astype(np.int64) for p in ptrs_list]
    for i, (n_out, f) in enumerate(zip(OUT_SIZES, FANINS)):
        seg = np.asarray(seg_list[i]).astype(np.int64)
        expected = np.repeat(np.arange(n_out, dtype=np.int64), f)
        assert np.array_equal(seg, expected), f"layer {i}: non-uniform segments"

    # remap ptr0 rows into the block layout [0, 0 | pos | neg]
    q0 = np.where(
        p0 < 2, p0, np.where(p0 % 2 == 0, 2 + (p0 - 2) // 2, 2 + n_vars + (p0 - 3) // 2)
    )

    k4 = np.arange(4)
    # phase 1: fused group g (t2 row g): 8 t0-rows q0[4*p1[2g]+k], q0[4*p1[2g+1]+k]
    a, b = p1[0::2], p1[1::2]
    g1 = np.concatenate(
        [q0[4 * a[:, None] + k4], q0[4 * b[:, None] + k4]], axis=1
    )  # [8192, 8]
    # phase 2: fused group h (out row h): 8 t2-rows p2[4*p3[2h]+k], p2[4*p3[2h+1]+k]
    c, d = p3[0::2], p3[1::2]
    g2 = np.concatenate(
        [p2[4 * c[:, None] + k4], p2[4 * d[:, None] + k4]], axis=1
    )  # [2048, 8]

    # dead-row elimination: phase 1 computes only t2 rows phase 2 reads
    used = np.unique(g2)  # sorted live t2 rows
    chunks = _plan(used.size)
    u_pad = sum(chunks)
    e_rows = sum(chunks[:-1])
    # pack phase-2 groups whose rows all precede the drain chunk; they
    # gather early, and out rows are emitted in permuted order
    early_g, late_g, early_rows = _pack_early(g2, e_rows)
    if early_g.size > N_OUT // 2:  # cap at half so both gathers are 4096-slot
        moved = early_g[N_OUT // 2 :]
        late_g = np.sort(np.concatenate([late_g, moved]))
        early_g = early_g[: N_OUT // 2]
        early_rows = np.array(sorted(set(g2[early_g].reshape(-1))), np.int64)
    n2a = -(-early_g.size // 4) * 4
    n2b = -(-(N_OUT - early_g.size) // 4) * 4
    _CACHE["plan"] = (chunks, n2a, n2b)
    late_rows = np.setdiff1d(used, early_rows)
    row_order = np.concatenate([early_rows, late_rows])  # compacted t2 order
    pos = np.zeros(N_MID, np.int64)
    pos[row_order] = np.arange(row_order.size)
    g2r = pos[g2]  # refs remapped to compacted row ids
    rows = np.concatenate([row_order, np.zeros(u_pad - row_order.size, np.int64)])
    g1c = g1[rows]  # [u_pad, 8] t0-indices per live (or pad) t2 row

    # device out row j holds original group out_perm[j]
    out_perm = np.concatenate([early_g, late_g])
    _CACHE["out_perm"] = out_perm
    ga = np.concatenate([g2r[early_g], np.zeros((n2a - early_g.size, 8), np.int64)])
    gb = np.concatenate([g2r[late_g], np.zeros((n2b - late_g.size, 8), np.int64)])

    idx_maps = {}
    r0 = 0
    for ci, gc in enumerate(chunks):
        gr = g1c[r0 : r0 + gc]
        hg = gc // 2
        idx_maps[f"idx1_{ci}"] = wrap128(gr[:hg].reshape(-1), gr[hg:].reshape(-1))
        r0 += gc
    idx_maps["idx2a"] = wrap128(
        ga[: n2a // 2].reshape(-1), ga[n2a // 2 :].reshape(-1)
    )
    idx_maps["idx2b"] = wrap128(
        gb[: n2b // 2].reshape(-1), gb[n2b // 2 :].reshape(-1)
    )

    batch = x.shape[1]
    bpc = batch // NCORES
    in_maps = []
    for i in range(NCORES):
        xt = np.ascontiguousarray(x[:, i * bpc : (i + 1) * bpc].T)  # [64, 4096]
        xv = np.concatenate([xt, xt], axis=0)  # [128, 4096], both copies
        in_maps.append({"xv": xv, **idx_maps})
    return in_maps


_CACHE = {}


def _get_nc():
    plan = _CACHE.get("plan", DEFAULT_PLAN)
    key = ("nc", plan)
    if key not in _CACHE:
        _CACHE[key] = build_nc(*plan)
    return _CACHE[key]


def kernel(x, ptrs0, seg0, ptrs1, seg1, ptrs2, seg2, ptrs3, seg3):
    from concourse.bass_utils import run_bass_kernel_spmd

    nc = _get_nc()
    in_maps = host_prep(
        x, [ptrs0, ptrs1, ptrs2, ptrs3], [seg0, seg1, seg2, seg3]
    )
    res = run_bass_kernel_spmd(nc, in_maps, core_ids=list(range(NCORES)))
    perm = _CACHE["out_perm"]
    full = np.empty((N_OUT, BATCH), np.float32)
    for i, r in enumerate(res.results):  # r["out"]: [64, n2a+n2b] permuted
        full[perm, i * B : (i + 1) * B] = r["out"][:, : perm.size].T
    return full
